# revision 16
# baseline (speedup 1.0000x reference)
"""W8A16 column-parallel linear for TRN2, 8 NeuronCores.

Computes y = x @ (qweight * w_scales).T + bias with
  x        [8, 1, 8192]  fp16
  qweight  [28672, 8192] int8 (per-row symmetric quant)
  w_scales [28672, 1]    fp16
  bias     [28672]       fp16
  y        [8, 1, 28672] fp16

Sharding: column-parallel — each of the 8 cores owns 3584 output rows
(qweight/w_scales/bias shard), x replicated. No collectives; outputs are
concatenated on the host.

Per-core kernel (v3): stream the int8 weight shard (transposed to
[K, Nshard] on host) from HBM in k-tile groups using the
"(g u p) n -> g p u n" descriptor layout (3584B runs — measured 366GB/s
vs 336 for partition-contiguous 14KB runs).  Two small (u=2) leading
groups start the convert pipeline ~3us earlier than a u=4 head; the
[2,1,1] tail drains it quickly after the last weight byte.  int8->fp16
conversion is free-dim split between VectorE 2x mode and ScalarE so the
pair sustains the HBM rate.  Matmuls accumulate [8, 512] PSUM chunks
(stationary x^T tile, moving weight tile) across 3 PE column groups
(tile_position; walrus rejects a 4th at col base 96).  Bias enters as
(sum x*q + b/s)*s via a K=1 ones^T @ (b/s) matmul that opens each PSUM
accumulation group.  Scales ride 3 small [8, NS] DMAs issued after the
last big weight group (the epilogue is their only consumer), saving
~0.35MB of stream vs a replicated-[72, NS] load.  Epilogue: VectorE
multiplies group 0 straight from PSUM while ScalarE copies groups 1/2
out of PSUM (fp16) for VectorE's 2x-mode fp16 multiplies; each group's
output DMA chases its multiply.
"""

import numpy as np

import concourse.bacc as bacc
import concourse.mybir as mybir
import concourse.tile as tile
from concourse.bass_utils import run_bass_kernel_spmd

B, S, K, N = 8, 1, 8192, 28672
M = B * S                 # 8 rows in the GEMM
NCORES = 8
NS = N // NCORES          # 3584 output rows per core
KT = K // 128             # 64 k-tiles
NCHUNK = NS // 512        # 7 psum chunks of 512
DVE_N = 2240              # free-dim split of the int8->fp16 conversion:
                          # VectorE takes [0, DVE_N), ScalarE the rest

# k-tile group sizes: small head so the first conversion starts early,
# uniform big groups in the stream, small tail for a fast drain.
GROUPS = [2, 2] + [4] * 14 + [2, 1, 1]
assert sum(GROUPS) == KT

_CACHE = {}

# chunk -> PE column-group (0,1,2 -> array cols 0-31/32-63/64-95). Three
# concurrent moving streams triple the PE's weight-streaming rate.
CHUNK_GRP = [0, 0, 0, 1, 1, 2, 2]
GRP_BASE = [32 * j for j in CHUNK_GRP]         # PSUM base partition per chunk
GRP_SPAN = {0: (0, 1536), 1: (1536, 2560), 2: (2560, 3584)}

# per-ktile matmul issue order rotates through the PE column groups so
# the three streams start back-to-back instead of blocking each other
ISSUE = [0, 3, 5, 1, 4, 6, 2]


def _build():
    nc = bacc.Bacc()
    xp = nc.declare_dram_parameter("x", [128, KT * M], mybir.dt.float16, isOutput=False)
    qp = nc.declare_dram_parameter("qt", [K, NS], mybir.dt.int8, isOutput=False)
    sp = nc.declare_dram_parameter("s", [8, NS], mybir.dt.float16, isOutput=False)
    bp = nc.declare_dram_parameter("b", [1, NS], mybir.dt.float16, isOutput=False)
    op = nc.declare_dram_parameter("out", [M, NS], mybir.dt.float16, isOutput=True)

    # whole-param rearranges per group size: these emit the efficient DMA
    # descriptor layout (slice-then-rearrange APs measurably degrade the
    # DMA stream, as does a partition-contiguous 14KB-run layout)
    qru = {
        usz: qp.rearrange("(g u p) n -> g p u n", u=usz, p=128)
        for usz in (1, 2, 4)
    }

    with tile.TileContext(nc) as tc:
        with (
            tc.tile_pool(name="const", bufs=1) as constp,
            tc.tile_pool(name="wq", bufs=6) as wqp,
            tc.tile_pool(name="wf", bufs=3) as wfp,
            tc.tile_pool(name="psum", bufs=1, space="PSUM") as psp,
            tc.tile_pool(name="outp", bufs=1) as outp,
        ):
            xsb = constp.tile([128, KT * M], mybir.dt.float16, tag="xsb")
            sb = constp.tile([72, NS], mybir.dt.float16, tag="sb")
            b1 = constp.tile([1, NS], mybir.dt.float16, tag="b1")
            ones = constp.tile([1, M], mybir.dt.float16, tag="ones")

            # first weight group ahead of the constants on the HWDGE queue:
            # the weight stream is the binding resource
            wq0 = wqp.tile([128, GROUPS[0], NS], mybir.dt.int8, tag="wq")
            nc.sync.dma_start(wq0[:], qru[GROUPS[0]][0])
            nc.sync.dma_start(xsb[:], xp[:])
            nc.sync.dma_start(b1[:], bp[:])
            nc.gpsimd.memset(ones[:], 1.0)

            # one PSUM allocation spanning 7 banks: chunk c lives at
            # columns [c*512, (c+1)*512) (bank-aligned), partition rows
            # 32*grp(c) .. +8
            psum = psp.tile([128, NS], mybir.dt.float32, tag="psum")
            # keep-alive target in the free 8th PSUM bank: tiny matmuls
            # spaced ~2.4us keep the PE_HAM activity window non-idle so
            # the PE clock gate never drops to 1.2 GHz mid-stream (a cold
            # PE falls behind the wf-buffer recycle and stalls the CASTs)
            psd = psp.tile([1, M], mybir.dt.float32, tag="psd")
            for c in ISSUE:
                lo = GRP_BASE[c]
                # bias row opens the accumulation group: psum = ones^T @ (b/s)
                nc.tensor.matmul(
                    psum[lo:lo + M, c * 512:(c + 1) * 512],
                    ones[:], b1[:, c * 512:(c + 1) * 512],
                    start=True, stop=False,
                    tile_position=(0, lo),
                )

            kt0 = 0
            for g, gu in enumerate(GROUPS):
                assert kt0 % gu == 0
                if g == 0:
                    wq = wq0
                else:
                    wq = wqp.tile([128, gu, NS], mybir.dt.int8, tag="wq")
                    nc.sync.dma_start(wq[:], qru[gu][kt0 // gu])
                if g == len(GROUPS) - 3:
                    # scales are only needed by the epilogue multiplies:
                    # issue them here so they land just before the drain
                    # without delaying any mid-stream weight group.  One
                    # replica per PE column group's PSUM partition base.
                    for j in range(3):
                        nc.sync.dma_start(sb[32 * j:32 * j + 8, :], sp[:])
                wf = wfp.tile([128, gu, NS], mybir.dt.float16, tag="wf")
                nc.vector.tensor_copy(wf[:, :, 0:DVE_N], wq[:, :, 0:DVE_N])
                nc.scalar.activation(
                    wf[:, :, DVE_N:NS], wq[:, :, DVE_N:NS],
                    mybir.ActivationFunctionType.Copy,
                )
                for u in range(gu):
                    kt = kt0 + u
                    last = kt == KT - 1
                    for c in ISSUE:
                        lo = GRP_BASE[c]
                        nc.tensor.matmul(
                            psum[lo:lo + M, c * 512:(c + 1) * 512],
                            xsb[:, kt * M:(kt + 1) * M],
                            wf[:, u, c * 512:(c + 1) * 512],
                            start=False, stop=last,
                            tile_position=(0, lo),
                        )
                    if u % 2 == 0:
                        # HAM keep-alive (see psd above)
                        nc.tensor.matmul(
                            psd[:], ones[:, 0:1], ones[:],
                            start=True, stop=True,
                        )
                kt0 += gu

            # tail: VectorE multiplies group 0 straight from PSUM (its
            # largest span) while ScalarE drains groups 1/2 out of PSUM
            # as fp16 for VectorE's fast 2x-mode fp16 multiplies.  Each
            # group's output DMA chases its multiply.
            osb = outp.tile([72, NS], mybir.dt.float16, tag="osb")
            ot = outp.tile([72, NS], mybir.dt.float16, tag="ot")
            for j in (1, 2):
                nlo, nhi = GRP_SPAN[j]
                plo = 32 * j
                nc.scalar.activation(
                    ot[plo:plo + M, nlo:nhi], psum[plo:plo + M, nlo:nhi],
                    mybir.ActivationFunctionType.Copy,
                )
            nc.vector.tensor_mul(
                osb[0:M, 0:1536], psum[0:M, 0:1536], sb[0:M, 0:1536],
            )
            nc.sync.dma_start(op[:, 0:1536], osb[0:M, 0:1536])
            for j in (1, 2):
                nlo, nhi = GRP_SPAN[j]
                plo = 32 * j
                nc.vector.tensor_mul(
                    osb[plo:plo + M, nlo:nhi],
                    ot[plo:plo + M, nlo:nhi],
                    sb[plo:plo + M, nlo:nhi],
                )
                nc.sync.dma_start(op[:, nlo:nhi], osb[plo:plo + M, nlo:nhi])

    nc.compile()
    return nc


def _get_nc():
    if "nc" not in _CACHE:
        _CACHE["nc"] = _build()
    return _CACHE["nc"]


def _prep_inputs(x, qweight, w_scales, bias):
    x2 = np.asarray(x, dtype=np.float16).reshape(M, K)
    # xsb[p, kt*M + m] = x[m, kt*128 + p]
    xsb = np.ascontiguousarray(
        x2.T.reshape(KT, 128, M).transpose(1, 0, 2).reshape(128, KT * M)
    )
    qweight = np.asarray(qweight)
    w_scales = np.asarray(w_scales, dtype=np.float16).reshape(N)
    bias = np.asarray(bias, dtype=np.float16).reshape(N)
    in_maps = []
    for c in range(NCORES):
        sl = slice(c * NS, (c + 1) * NS)
        qt = np.ascontiguousarray(qweight[sl, :].T)          # [K, NS] int8
        s8 = np.ascontiguousarray(
            np.broadcast_to(w_scales[sl].reshape(1, NS), (8, NS))
        )
        # bias enters the PSUM accumulation before the scale multiply, so
        # pre-divide: out = (sum x*q + b/s) * s
        bos = (bias[sl].astype(np.float32)
               / w_scales[sl].astype(np.float32)).astype(np.float16)
        b1 = np.ascontiguousarray(bos.reshape(1, NS))         # [1, NS] fp16
        in_maps.append({"x": xsb, "qt": qt, "s": s8, "b": b1})
    return in_maps


def _run(x, qweight, w_scales, bias, trace=False):
    nc = _get_nc()
    in_maps = _prep_inputs(x, qweight, w_scales, bias)
    res = run_bass_kernel_spmd(
        nc, in_maps, core_ids=list(range(NCORES)), trace=trace
    )
    y = np.concatenate(
        [np.asarray(res.results[c]["out"]) for c in range(NCORES)], axis=1
    )
    return y.reshape(B, S, N).astype(np.float16), res


def kernel(x, qweight, w_scales, bias):
    y, _ = _run(x, qweight, w_scales, bias, trace=False)
    return y


def kernel_traced(x, qweight, w_scales, bias):
    """Like kernel() but also returns the BassKernelResults (exec_time_ns)."""
    return _run(x, qweight, w_scales, bias, trace=True)
